# revision 10
# baseline (speedup 1.0000x reference)
"""Trainium2 Bass kernel for GrowingFieldV2 GNN message passing.

Data-parallel over batch: 8 NeuronCores, each processing a 1024-row shard
of x. Small [500,*] parameters (positions/features/weights) are replicated
and the [500,500] connectivity matrix is computed redundantly on every core.

Per-core device program:
  phase 0: build row-normalized conn C = 0.5/rowsum * sym  (bf16)
           and W'' = (I+C)^T (I+C)^T (ow*og)  [500,10]
  phase 1: actT0 = (x @ iw.T).T * input_gate + bias        (bf16 matmuls)
  phase 2: ONE explicit message passing step
           act1 = relu(actT0 + C @ actT0)    (relu via Scalar engine)
           (iterations 2+3 are affine -- relu/min(50) provably inactive --
            so they are folded into the output weights W'')
  phase 3: yT = W''^T @ act1  -> [10, 1024]

HBM layouts are k-major packed on the host so every DMA moves large
contiguous per-partition lines (the baseline's row-interleaved DMAs ran
at ~140 GB/s; these run near the HBM roofline).
"""

import sys

for _p in ("/opt/trn_rl_repo",):
    if _p not in sys.path:
        sys.path.insert(0, _p)

import numpy as np

N = 500            # neurons
IN = 3072          # input size
FD = 64            # feature dim
OUT = 10           # output size
B = 8192           # full batch
NCORES = 8
BS = B // NCORES   # 1024 per-core batch shard
RADIUS = 20.0
VOL = 100.0

NT = 4             # neuron tiles
NP = N // NT       # 125 neurons per tile
KT = IN // 128     # 24 contraction tiles for phase 1
NCH = 2            # batch chunks of 512 (PSUM bank width)
CH = BS // NCH     # 512

XGROUPS = [2, 6, 8, 8]     # k-tiles per x DMA group
IWGROUPS = [4, 10, 10]     # k-tiles per iw DMA group
SPW = 14                   # packed small-param width per m-tile: 3 pos + 10 ow + 1 bias

_CACHE = {}


def _build():
    import concourse.bacc as bacc
    import concourse.tile as tile
    import concourse.bass as bass
    import concourse.mybir as mybir

    f32 = mybir.dt.float32
    bf16 = mybir.dt.bfloat16
    AF = mybir.ActivationFunctionType
    ALU = mybir.AluOpType
    PSUM = bass.MemorySpace.PSUM

    nc = bacc.Bacc("TRN2", target_bir_lowering=False, debug=False,
                   num_devices=NCORES)

    xk_d = nc.dram_tensor("xk", [128, KT * BS], bf16, kind="ExternalInput").ap()
    iwk_d = nc.dram_tensor("iwk", [128, KT * N], bf16, kind="ExternalInput").ap()
    posT_d = nc.dram_tensor("posT", [3, N], f32, kind="ExternalInput").ap()
    featT_d = nc.dram_tensor("featT", [FD, N], f32, kind="ExternalInput").ap()
    spk_d = nc.dram_tensor("spk", [NP, NT * SPW], f32, kind="ExternalInput").ap()
    yT_d = nc.dram_tensor("yT", [OUT, BS], f32, kind="ExternalOutput").ap()

    with tile.TileContext(nc) as tc:
        with (
            tc.tile_pool(name="wts", bufs=1) as wts,
            tc.tile_pool(name="acts", bufs=2) as acts,
            tc.tile_pool(name="stage", bufs=1) as stage,
            tc.tile_pool(name="cwork", bufs=2) as cwork,
            tc.tile_pool(name="small", bufs=1) as small,
            tc.tile_pool(name="ps", bufs=1, space=PSUM) as ps,
        ):
            # ---------- parameter DMAs (scalar queue, issued first) ----------
            posT_sb = small.tile([3, N], f32, tag="posT")
            nc.scalar.dma_start(out=posT_sb[:], in_=posT_d[:])
            featT_sb = small.tile([FD, N], f32, tag="featT")
            nc.scalar.dma_start(out=featT_sb[:], in_=featT_d[:])
            spk_sb = small.tile([NP, NT * SPW], f32, tag="spk")
            nc.scalar.dma_start(out=spk_sb[:], in_=spk_d[:])

            # ---------- bulk DMAs (sync queue): k-major contiguous ----------
            xg_sb = []
            iwg_sb = []
            xoffs = np.cumsum([0] + XGROUPS)
            iwoffs = np.cumsum([0] + IWGROUPS)
            for gi in range(max(len(XGROUPS), len(IWGROUPS))):
                if gi < len(XGROUPS):
                    g = XGROUPS[gi]
                    t = wts.tile([128, g * BS], bf16, tag=f"xg{gi}")
                    nc.sync.dma_start(
                        out=t[:],
                        in_=xk_d[:, xoffs[gi] * BS:xoffs[gi + 1] * BS])
                    xg_sb.append(t)
                if gi < len(IWGROUPS):
                    g = IWGROUPS[gi]
                    t = wts.tile([128, g * N], bf16, tag=f"iwg{gi}")
                    nc.sync.dma_start(
                        out=t[:],
                        in_=iwk_d[:, iwoffs[gi] * N:iwoffs[gi + 1] * N])
                    iwg_sb.append(t)

            def x_slice(k, c):
                gi = int(np.searchsorted(xoffs, k, side="right")) - 1
                a = k - xoffs[gi]
                return xg_sb[gi][:, a * BS + c * CH:a * BS + (c + 1) * CH]

            def iw_slice(k, m):
                gi = int(np.searchsorted(iwoffs, k, side="right")) - 1
                a = k - iwoffs[gi]
                return iwg_sb[gi][:, a * N + m * NP:a * N + (m + 1) * NP]

            # ---------- preamble scalars/gates ----------
            # clip positions into the volume (per reference)
            posTc = small.tile([3, N], f32, tag="posTc")
            nc.vector.tensor_scalar(out=posTc[:], in0=posT_sb[:],
                                    scalar1=0.1, scalar2=VOL - 0.1,
                                    op0=ALU.max, op1=ALU.min)
            posTcc = small.tile([3, N], f32, tag="posTcc")
            nc.vector.tensor_scalar(out=posTcc[:], in0=posTc[:],
                                    scalar1=50.0, scalar2=None,
                                    op0=ALU.subtract)
            pos2 = small.tile([3, N], f32, tag="pos2")
            nc.vector.tensor_mul(pos2[:], posTcc[:], posTcc[:])
            feat2 = small.tile([FD, N], f32, tag="feat2")
            nc.vector.tensor_mul(feat2[:], featT_sb[:], featT_sb[:])

            posx_m = []   # clipped x-coordinate columns [125,1]
            for m in range(NT):
                pc = small.tile([NP, 1], f32, tag=f"posx{m}")
                nc.vector.tensor_scalar(out=pc[:],
                                        in0=spk_sb[:, m * SPW:m * SPW + 1],
                                        scalar1=0.1, scalar2=VOL - 0.1,
                                        op0=ALU.max, op1=ALU.min)
                posx_m.append(pc)

            ones3 = small.tile([3, 1], f32, tag="ones3")
            nc.vector.memset(ones3[:], 1.0)
            ones64 = small.tile([FD, 1], f32, tag="ones64")
            nc.vector.memset(ones64[:], 1.0)
            ones1 = small.tile([1, NP], f32, tag="ones1")
            nc.vector.memset(ones1[:], 1.0)
            neg2_row = small.tile([1, 1], f32, tag="neg2row")
            nc.vector.memset(neg2_row[:], -2.0)
            neg2_col = small.tile([NP, 1], f32, tag="neg2col")
            nc.vector.memset(neg2_col[:], -2.0)

            # --- ACT batch 1: all Exp ops that only need positions ---
            igrow = small.tile([1, N], f32, tag="igrow")
            nc.scalar.activation(igrow[:], posTc[0:1, :], AF.Exp, scale=-2.0 / VOL)
            ogrow = small.tile([1, N], f32, tag="ogrow")
            nc.scalar.activation(ogrow[:], posTc[0:1, :], AF.Exp,
                                 scale=2.0 / VOL, bias=neg2_row[:])
            ie_m = []
            oe_m = []
            for m in range(NT):
                ie = small.tile([NP, 1], f32, tag=f"igexp{m}")
                nc.scalar.activation(ie[:], posx_m[m][:], AF.Exp, scale=-2.0 / VOL)
                ie_m.append(ie)
                oe = small.tile([NP, 1], f32, tag=f"ogexp{m}")
                nc.scalar.activation(oe[:], posx_m[m][:], AF.Exp,
                                     scale=2.0 / VOL, bias=neg2_col[:])
                oe_m.append(oe)

            igsum = small.tile([1, 1], f32, tag="igsum")
            nc.vector.reduce_sum(igsum[:], igrow[:], axis=mybir.AxisListType.X)
            ogsum = small.tile([1, 1], f32, tag="ogsum")
            nc.vector.reduce_sum(ogsum[:], ogrow[:], axis=mybir.AxisListType.X)

            # ---------- tiny PE matmuls ----------
            r2_ps = ps.tile([1, N], f32, tag="ps0")
            nc.tensor.matmul(r2_ps[:], ones3[:], pos2[:], start=True, stop=True)
            r2row = small.tile([1, N], f32, tag="r2row")
            nc.vector.tensor_copy(r2row[:], r2_ps[:])

            f2_ps = ps.tile([1, N], f32, tag="ps1")
            nc.tensor.matmul(f2_ps[:], ones64[:], feat2[:], start=True, stop=True)
            # 1/sqrt via DVE reciprocal then ACT Sqrt (keeps Sqrt in batch 2)
            f2r = small.tile([1, N], f32, tag="f2r")
            nc.vector.tensor_scalar(out=f2r[:], in0=f2_ps[:], scalar1=1e-12,
                                    scalar2=None, op0=ALU.max)
            f2rec = small.tile([1, N], f32, tag="f2rec")
            nc.vector.reciprocal(f2rec[:], f2r[:])
            # rnrow = 1/max(||f||, 1e-6)  (Sqrt lands in ACT batch 2)
            rnrow = small.tile([1, N], f32, tag="rnrow")
            nc.scalar.activation(rnrow[:], f2rec[:], AF.Sqrt)

            igs_ps = ps.tile([NP, 1], f32, tag="ps2", name="igs_ps")
            nc.tensor.matmul(igs_ps[:], ones1[:], igsum[:], start=True, stop=True)
            igsum2 = small.tile([NP, 1], f32, tag="igsum2")
            nc.vector.tensor_scalar(out=igsum2[:], in0=igs_ps[:], scalar1=1e-6,
                                    scalar2=None, op0=ALU.add)
            igb = small.tile([NP, 1], f32, tag="igb")
            nc.vector.reciprocal(igb[:], igsum2[:])

            ogs_ps = ps.tile([NP, 1], f32, tag="ps3", name="ogs_ps")
            nc.tensor.matmul(ogs_ps[:], ones1[:], ogsum[:], start=True, stop=True)
            ogsum2 = small.tile([NP, 1], f32, tag="ogsum2")
            nc.vector.tensor_scalar(out=ogsum2[:], in0=ogs_ps[:], scalar1=1e-6,
                                    scalar2=None, op0=ALU.add)
            ogb = small.tile([NP, 1], f32, tag="ogb")
            nc.vector.reciprocal(ogb[:], ogsum2[:])

            # broadcast rows to [125, N] tiles via PE ones-matmul
            r2b_ps = ps.tile([NP, N], f32, tag="ps0", name="r2b_ps")
            nc.tensor.matmul(r2b_ps[:], ones1[:], r2row[:], start=True, stop=True)
            r2b = small.tile([NP, N], f32, tag="r2b")
            nc.vector.tensor_copy(r2b[:], r2b_ps[:])
            rnb_ps = ps.tile([NP, N], f32, tag="ps1", name="rnb_ps")
            nc.tensor.matmul(rnb_ps[:], ones1[:], rnrow[:], start=True, stop=True)
            rnb = small.tile([NP, N], f32, tag="rnb")
            nc.vector.tensor_copy(rnb[:], rnb_ps[:])

            # row -> column slices [125,1] via small DMAs
            rn_col = []
            r2_col = []
            for m in range(NT):
                rc = small.tile([NP, 1], f32, tag=f"rncol{m}")
                nc.sync.dma_start(out=rc[:], in_=rnrow[0:1, m * NP:(m + 1) * NP])
                rn_col.append(rc)
                r2c = small.tile([NP, 1], f32, tag=f"r2col{m}")
                nc.sync.dma_start(out=r2c[:], in_=r2row[0:1, m * NP:(m + 1) * NP])
                r2_col.append(r2c)

            # per-tile gate columns + output-weight columns
            gate_m = []
            v0_m = []
            for m in range(NT):
                g = small.tile([NP, 1], f32, tag=f"gate{m}")
                nc.vector.tensor_mul(g[:], ie_m[m][:], igb[:])
                gate_m.append(g)
                og = small.tile([NP, 1], f32, tag=f"og{m}")
                nc.vector.tensor_mul(og[:], oe_m[m][:], ogb[:])
                v0 = small.tile([NP, OUT], f32, tag=f"v0_{m}")
                nc.vector.tensor_scalar(
                    out=v0[:], in0=spk_sb[:, m * SPW + 3:m * SPW + 3 + OUT],
                    scalar1=og[:], scalar2=None, op0=ALU.mult)
                v0_m.append(v0)

            # ---------- connectivity: grams then staged epilogues ----------
            gf_sb = []
            sq_m = []
            for m in range(NT):
                gfp = ps.tile([NP, N], f32, tag=f"ps{m}")
                nc.tensor.matmul(gfp[:], featT_sb[:, m * NP:(m + 1) * NP],
                                 featT_sb[:], start=True, stop=True)
                gf = stage.tile([NP, N], bf16, tag=f"gf{m}")
                nc.vector.tensor_copy(gf[:], gfp[:])
                gf_sb.append(gf)

                gp = ps.tile([NP, N], f32, tag=f"ps{m}")
                nc.tensor.matmul(gp[:], posTcc[:, m * NP:(m + 1) * NP],
                                 posTcc[:], start=True, stop=True)
                # sq = max(-2G + r2_j + r2_i, 0)
                sq1 = cwork.tile([NP, N], f32, tag="sq1")
                nc.vector.scalar_tensor_tensor(out=sq1[:], in0=gp[:],
                                               scalar=-2.0, in1=r2b[:],
                                               op0=ALU.mult, op1=ALU.add)
                sq = stage.tile([NP, N], f32, tag=f"sq{m}")
                nc.vector.tensor_scalar(out=sq[:], in0=sq1[:],
                                        scalar1=r2_col[m][:], scalar2=0.0,
                                        op0=ALU.add, op1=ALU.max)
                sq_m.append(sq)

            # ACT batch 2 (Sqrt): all distance tiles back-to-back
            dist_m = []
            for m in range(NT):
                dist = stage.tile([NP, N], f32, tag=f"dist{m}")
                nc.scalar.activation(dist[:], sq_m[m][:], AF.Sqrt)
                dist_m.append(dist)
            # ACT batch 3 (Exp): attenuation tiles back-to-back
            att0_m = []
            for m in range(NT):
                att0 = stage.tile([NP, N], f32, tag=f"att0{m}")
                nc.scalar.activation(att0[:], dist_m[m][:], AF.Exp,
                                     scale=-1.0 / RADIUS)
                att0_m.append(att0)
            # preload the Relu table (used by the phase-2 epilogue much later)
            relu_dummy = small.tile([NP, 1], f32, tag="reludummy")
            nc.scalar.activation(relu_dummy[:], posx_m[0][:], AF.Relu)

            # bf16 conn tiles stay UNNORMALIZED (symmetric!) -- row scale
            # rh = 0.5/(rowsum+1e-6) is applied per output partition instead.
            # Symmetry makes lhsT slicing give C@act as well as C^T@v.
            conn_m = []
            rs_col = []
            for m in range(NT):
                attm = cwork.tile([NP, N], f32, tag="attm")
                nc.vector.scalar_tensor_tensor(out=attm[:], in0=dist_m[m][:],
                                               scalar=RADIUS, in1=att0_m[m][:],
                                               op0=ALU.is_lt, op1=ALU.mult)
                attz = cwork.tile([NP, N], f32, tag="attz")
                nc.gpsimd.affine_select(out=attz[:], in_=attm[:],
                                        pattern=[[1, N]],
                                        compare_op=ALU.not_equal, fill=0.0,
                                        base=-m * NP, channel_multiplier=-1)
                # feature similarity -> 0.5 + 0.5*cos
                t1 = cwork.tile([NP, N], f32, tag="t1")
                nc.vector.scalar_tensor_tensor(out=t1[:], in0=gf_sb[m][:],
                                               scalar=rn_col[m][:], in1=rnb[:],
                                               op0=ALU.mult, op1=ALU.mult)
                fs = cwork.tile([NP, N], f32, tag="fs")
                nc.vector.tensor_scalar(out=fs[:], in0=t1[:], scalar1=0.5,
                                        scalar2=0.5, op0=ALU.mult, op1=ALU.add)
                cb = wts.tile([NP, N], bf16, tag=f"conn{m}")
                rsc = small.tile([NP, 1], f32, tag=f"rscol{m}")
                nc.vector.scalar_tensor_tensor(out=cb[:], in0=fs[:],
                                               scalar=1.0, in1=attz[:],
                                               op0=ALU.mult, op1=ALU.mult,
                                               accum_out=rsc[:])
                conn_m.append(cb)
                rs_col.append(rsc)

            rh_m = []
            u1_m = []
            for m in range(NT):
                rsc2 = small.tile([NP, 1], f32, tag=f"rsc2{m}")
                nc.vector.tensor_scalar(out=rsc2[:], in0=rs_col[m][:],
                                        scalar1=1e-6, scalar2=None, op0=ALU.add)
                rrec = small.tile([NP, 1], f32, tag=f"rrec{m}")
                nc.vector.reciprocal(rrec[:], rsc2[:])
                rh = small.tile([NP, 1], f32, tag=f"rhalf{m}")
                nc.vector.tensor_scalar(out=rh[:], in0=rrec[:], scalar1=0.5,
                                        scalar2=None, op0=ALU.mult)
                rh_m.append(rh)
                # u1 = rh * v0  (bf16)  -- first application of diag(rh)
                u1 = small.tile([NP, OUT], bf16, tag=f"u1_{m}")
                nc.vector.tensor_scalar(out=u1[:], in0=v0_m[m][:],
                                        scalar1=rh[:], scalar2=None,
                                        op0=ALU.mult)
                u1_m.append(u1)

            # ---------- phase 1: actT0 = (x @ iw.T).T * gate + bias ----------
            ps_act = [ps.tile([NP, BS], f32, tag=f"ps{m}", name=f"psact{m}")
                      for m in range(NT)]
            for k in range(KT):
                for m in range(NT):
                    for c in range(NCH):
                        nc.tensor.matmul(
                            ps_act[m][:, c * CH:(c + 1) * CH],
                            iw_slice(k, m), x_slice(k, c),
                            start=(k == 0), stop=(k == KT - 1))

            act0 = []
            for m in range(NT):
                a = acts.tile([NP, BS], bf16, tag=f"act{m}")
                nc.vector.tensor_scalar(
                    out=a[:], in0=ps_act[m][:], scalar1=gate_m[m][:],
                    scalar2=spk_sb[:, m * SPW + 13:m * SPW + 14],
                    op0=ALU.mult, op1=ALU.add)
                act0.append(a)

            # ---------- phase 2: one explicit message-passing step ----------
            ps_mp = [ps.tile([NP, BS], f32, tag=f"ps{m}", name=f"psmp{m}")
                     for m in range(NT)]
            for m in range(NT):
                for a in range(NT):
                    for c in range(NCH):
                        nc.tensor.matmul(
                            ps_mp[m][:, c * CH:(c + 1) * CH],
                            conn_m[a][:, m * NP:(m + 1) * NP],
                            act0[a][:, c * CH:(c + 1) * CH],
                            start=(a == 0), stop=(a == NT - 1))
            act1 = []
            for m in range(NT):
                # upd = rh * msg + act0   (pre-relu; min(50) provably inactive)
                upd = cwork.tile([NP, BS], f32, tag="upd")
                nc.vector.scalar_tensor_tensor(
                    out=upd[:], in0=ps_mp[m][:], scalar=rh_m[m][:],
                    in1=act0[m][:], op0=ALU.mult, op1=ALU.add)
                a2 = acts.tile([NP, BS], bf16, tag=f"act{m}")
                nc.scalar.activation(a2[:], upd[:], AF.Relu)
                act1.append(a2)

            # ---------- W'' = M^T M^T v0 with M = I + diag(rh) sym --------
            # M^T v = v + sym^T diag(rh) v = v + sym (rh*v)   (sym symmetric)
            w1_ps = ps.tile([NP, NT * OUT], f32, tag="ps0", name="w1_ps")
            for mo in range(NT):
                for a in range(NT):
                    nc.tensor.matmul(
                        w1_ps[:, mo * OUT:(mo + 1) * OUT],
                        conn_m[a][:, mo * NP:(mo + 1) * NP], u1_m[a][:],
                        start=(a == 0), stop=(a == NT - 1))
            # v1 = v0 + sym@u1 ; u2 = rh*v1 = u1 + rh*(sym@u1)
            v1_m = []
            u2_m = []
            for mo in range(NT):
                v1 = small.tile([NP, OUT], f32, tag=f"v1_{mo}")
                nc.vector.scalar_tensor_tensor(
                    out=v1[:], in0=w1_ps[:, mo * OUT:(mo + 1) * OUT],
                    scalar=1.0, in1=v0_m[mo][:], op0=ALU.mult, op1=ALU.add)
                v1_m.append(v1)
                u2 = small.tile([NP, OUT], bf16, tag=f"u2_{mo}")
                nc.vector.scalar_tensor_tensor(
                    out=u2[:], in0=w1_ps[:, mo * OUT:(mo + 1) * OUT],
                    scalar=rh_m[mo][:], in1=u1_m[mo][:],
                    op0=ALU.mult, op1=ALU.add)
                u2_m.append(u2)
            w2_ps = ps.tile([NP, NT * OUT], f32, tag="ps1", name="w2_ps")
            for mo in range(NT):
                for a in range(NT):
                    nc.tensor.matmul(
                        w2_ps[:, mo * OUT:(mo + 1) * OUT],
                        conn_m[a][:, mo * NP:(mo + 1) * NP], u2_m[a][:],
                        start=(a == 0), stop=(a == NT - 1))
            wtil_m = []
            for mo in range(NT):
                wt = small.tile([NP, OUT], bf16, tag=f"wtil{mo}")
                nc.vector.scalar_tensor_tensor(
                    out=wt[:], in0=w2_ps[:, mo * OUT:(mo + 1) * OUT],
                    scalar=1.0, in1=v1_m[mo][:], op0=ALU.mult, op1=ALU.add)
                wtil_m.append(wt)

            # ---------- phase 3: output ----------
            ps_y = ps.tile([OUT, BS], f32, tag="ps2", name="ps_y")
            for c in range(NCH):
                for a in range(NT):
                    nc.tensor.matmul(ps_y[:, c * CH:(c + 1) * CH],
                                     wtil_m[a][:],
                                     act1[a][:, c * CH:(c + 1) * CH],
                                     start=(a == 0), stop=(a == NT - 1))
                y_sb = small.tile([OUT, CH], f32, tag=f"ysb{c}")
                nc.vector.tensor_copy(y_sb[:], ps_y[:, c * CH:(c + 1) * CH])
                nc.sync.dma_start(out=yT_d[:, c * CH:(c + 1) * CH], in_=y_sb[:])

    nc.compile()
    return nc


def _get_nc():
    if "nc" not in _CACHE:
        _CACHE["nc"] = _build()
    return _CACHE["nc"]


def _pack_host(positions, input_weights, features, output_weights, biases):
    """Host-side packing of the replicated parameter tensors."""
    import concourse.mybir as mybir

    bf16_np = mybir.dt.np(mybir.dt.bfloat16)

    pos = np.ascontiguousarray(positions, dtype=np.float32)
    posT = np.ascontiguousarray(pos.T)                       # [3, N]
    featT = np.ascontiguousarray(
        np.asarray(features, dtype=np.float32).T)            # [FD, N]

    # iw k-major: iwk[p, k*N + n] = input_weights[n, k*128 + p]
    iwT = np.asarray(input_weights, dtype=np.float32).T      # [IN, N]
    iwk = np.ascontiguousarray(
        iwT.reshape(KT, 128, N).transpose(1, 0, 2).reshape(128, KT * N)
    ).astype(bf16_np)

    # packed per-m small params: [125, NT*(3 pos + 10 ow + 1 bias)]
    ow = np.asarray(output_weights, dtype=np.float32)
    bias = np.asarray(biases, dtype=np.float32).reshape(N, 1)
    spk = np.empty((NP, NT * SPW), dtype=np.float32)
    for m in range(NT):
        rows = slice(m * NP, (m + 1) * NP)
        spk[:, m * SPW:m * SPW + 3] = pos[rows]
        spk[:, m * SPW + 3:m * SPW + 13] = ow[rows]
        spk[:, m * SPW + 13:m * SPW + 14] = bias[rows]

    return posT, featT, iwk, spk


def _run(x, positions, input_weights, features, output_weights, biases,
         trace=False):
    from concourse.bass_utils import run_bass_kernel_spmd
    import concourse.mybir as mybir

    bf16_np = mybir.dt.np(mybir.dt.bfloat16)

    nc = _get_nc()
    posT, featT, iwk, spk = _pack_host(
        positions, input_weights, features, output_weights, biases)

    x = np.asarray(x, dtype=np.float32)
    in_maps = []
    for c in range(NCORES):
        xs = x[c * BS:(c + 1) * BS, :].T                     # [IN, BS]
        xk = np.ascontiguousarray(
            xs.reshape(KT, 128, BS).transpose(1, 0, 2).reshape(128, KT * BS)
        ).astype(bf16_np)
        in_maps.append({
            "xk": xk, "iwk": iwk, "posT": posT, "featT": featT, "spk": spk,
        })

    res = run_bass_kernel_spmd(nc, in_maps, list(range(NCORES)), trace=trace)
    y = np.empty((B, OUT), dtype=np.float32)
    for c in range(NCORES):
        y[c * BS:(c + 1) * BS, :] = res.results[c]["yT"].T
    return y, res


def kernel(x, positions, input_weights, features, output_weights, biases):
    y, _ = _run(x, positions, input_weights, features, output_weights, biases)
    return y
